# revision 15
# baseline (speedup 1.0000x reference)
"""Trainium2 Bass kernel for the ControlUnit problem.

Computation (per batch b):
    cq      = concat([control_state, question])            # [2D]
    cq_proj = cq @ W_cq + b_cq + step_emb[step]            # [D]
    qw      = cq_proj * W_attn                             # [D]
    logits  = context[b] @ qw  (+ b_attn, softmax-invariant, dropped)
    w       = softmax(where(mask, logits, -1e4))           # [L]
    out[b]  = w @ context[b]                               # [D]

Sharding: data-parallel over batch across 8 NeuronCores (8 batches/core);
params replicated.

Per-core kernel (all matmul operands bf16, fp32 PSUM accumulation):
  The host ships context twice: natural layout [l, d] (for the weighted
  sum, which contracts l) and pre-transposed [d, l] (for the logits,
  which contract d).  Shipping ctxT costs +16.8MB DMA but removes the
  512 PE transpose matmuls + 128 PSUM->SBUF drain copies an on-chip
  transpose needs; the per-exec wall time is dominated by instruction
  dispatch/sequencer occupancy, so trading DMA bytes for a ~5x smaller
  instruction stream is a net win on this runtime (measured).

  Phase 1: cq_proj matmul (bias folded via augmented ones column /
           bias row; [8,512] PSUM chunk accumulators), qw = cq_proj *
           W_attn, PE-transpose qw so d sits on partitions.
  Phase 2 (per batch): logits = one bf16 mask-bias matmul + 16 d-chunk
           matmuls of qwT against the shipped ctxT; one fused exp over
           [8,512] (no max subtraction - logits are ~N(0,1)); a
           predicated copy keeps the true softmax-numerator row; 4 PE
           transposes put the exp-weights l-on-partitions; the weighted
           sum accumulates in [8,512] PSUM chunks, DVE-added into an
           SBUF accumulator.  Denominators from one reduce_sum/
           reciprocal; final scale + output DMA run per 512-chunk.
"""
import numpy as np
import ml_dtypes
from contextlib import ExitStack

import concourse.bass as bass
import concourse.tile as tile
from concourse import bacc, mybir
from concourse.bass_utils import run_bass_kernel_spmd

F32 = mybir.dt.float32
BF16 = mybir.dt.bfloat16

N_CORES = 8
B, L, D = 64, 512, 2048


def build_nc(b_c, l, d, n_cores):
    """Build + compile the per-core Bass program (SPMD: same program on all
    cores, different data)."""
    d2a = 2 * d + 128          # augmented contraction dim (bias row block)
    KC = 5                     # k-tiles of the cq_proj matmul per core
    # (d2a = 4224 padded to KC*128*n_cores = 5120 on the host; each core
    # contracts its own 640-row W slab against all B batches, and a
    # ReduceScatter sums the partials and hands each core its 8 rows)
    LT = l // 128              # l-tiles per batch
    DC = d // 128              # 128-wide d-chunks
    NN = d // 512              # 512-wide n-chunks
    Ball = b_c * n_cores

    nc = bacc.Bacc("TRN2", target_bir_lowering=False, debug=False,
                   num_devices=n_cores)

    ctx_d = nc.dram_tensor("ctx", [b_c, l, d], BF16, kind="ExternalInput")
    ctxT_d = nc.dram_tensor("ctxT", [b_c, DC, 128, l], BF16,
                            kind="ExternalInput")
    BLOB = 128 + l + KC * Ball
    blob_d = nc.dram_tensor("blob", [128, BLOB], BF16, kind="ExternalInput")
    w_d = nc.dram_tensor("w_aug", [KC * 128, d], BF16, kind="ExternalInput")
    qw64_d = nc.dram_tensor("qw64", [Ball, d], F32, kind="Internal")
    qwrs_d = nc.dram_tensor("qwrs", [b_c, d], F32, kind="Internal")
    out_d = nc.dram_tensor("out", [b_c, d], F32, kind="ExternalOutput")

    Exp = mybir.ActivationFunctionType.Exp

    with tile.TileContext(nc) as tc:
        with ExitStack() as ctx:
            const = ctx.enter_context(tc.tile_pool(name="const", bufs=1))
            wpool = ctx.enter_context(tc.tile_pool(name="wpool", bufs=4))
            natpool = ctx.enter_context(tc.tile_pool(name="natpool", bufs=3))
            ctpool = ctx.enter_context(tc.tile_pool(name="ctpool", bufs=3))
            ps_lg_p = ctx.enter_context(tc.tile_pool(name="ps_lg_p", bufs=2,
                                                     space="PSUM"))
            ps_out_p = ctx.enter_context(tc.tile_pool(name="ps_out_p", bufs=2,
                                                      space="PSUM"))
            lgpool = ctx.enter_context(tc.tile_pool(name="lgpool", bufs=2))
            ps_tr_p = ctx.enter_context(tc.tile_pool(name="ps_tr_p", bufs=2,
                                                     space="PSUM"))

            # ---- constants / persistent tiles ----
            blob_sb = const.tile([128, BLOB], BF16)
            nc.sync.dma_start(blob_sb[:, :], blob_d[:, :])
            idb = blob_sb[:, 0:128]
            mask_sb = blob_sb[:, 128:128 + l]
            cqT_sb = blob_sb[:, 128 + l:]

            qw_sb = const.tile([b_c, d], BF16)
            qwT_sb = const.tile([128, DC * b_c], BF16)
            W8 = const.tile([128, b_c, LT, b_c], BF16)
            nc.gpsimd.memset(W8[:, :, :, :], 0.0)
            # per-batch exp row-sums land in column b; the true softmax
            # denominator for batch b is the diagonal element [b, b]
            ssum_all = const.tile([b_c, b_c], F32)
            ssum_sb = const.tile([b_c, 1], F32)
            sinv_sb = const.tile([b_c, 1], F32)
            out_sb = const.tile([b_c, d], F32)

            acc_sb = const.tile([b_c, d], F32)

            # ---- phase 2 helpers ----
            nat4s = {}
            ctbs = {}

            def load(b):
                # shipped pre-transposed context: [128 d-part, DC, l]
                ctb = ctpool.tile([128, DC, l], BF16, name="ctb")
                nc.sync.dma_start(
                    ctb[:, :, :],
                    ctxT_d[b].rearrange("c p n -> p c n"))
                ctbs[b] = ctb
                nat4 = natpool.tile([128, LT, d], BF16, name="nat4")
                nc.sync.dma_start(
                    nat4[:, :, :],
                    ctx_d[b].rearrange("(a p) n -> p a n", p=128))
                nat4s[b] = nat4

            def attend(b):
                nats = [nat4s[b][:, i, :] for i in range(LT)]
                ctb = ctbs[b]
                # logits for all queries vs this batch's context; row b is
                # real.  First matmul adds the -10000 mask bias.
                ps_lg = ps_lg_p.tile([b_c, l], F32, name="ps_lg")
                nc.tensor.matmul(
                    ps_lg[:, :],
                    lhsT=idb[:, 0:b_c],
                    rhs=mask_sb[:, :],
                    start=True, stop=False,
                )
                for j in range(DC):
                    nc.tensor.matmul(
                        ps_lg[:, :],
                        lhsT=qwT_sb[:, j * b_c:(j + 1) * b_c],
                        rhs=ctb[:, j, :],
                        start=False,
                        stop=(j == DC - 1),
                    )
                pm_sb = lgpool.tile([b_c, l], BF16, name="pm_sb")
                nc.scalar.activation(pm_sb[:, :], ps_lg[:, :], Exp,
                                     accum_out=ssum_all[:, b:b + 1])
                if b == b_c - 1:
                    # softmax denominators: diagonal of ssum_all (must be
                    # emitted before the per-chunk scale below so Tile
                    # orders the deps right)
                    nc.vector.tensor_mul(ssum_all[:, :], ssum_all[:, :],
                                         idb[0:b_c, 0:b_c])
                    nc.vector.reduce_sum(ssum_sb[:, :], ssum_all[:, :],
                                         axis=mybir.AxisListType.X)
                    nc.vector.reciprocal(sinv_sb[:, :], ssum_sb[:, :])

                # transpose exp-weights to l-on-partitions, drop into W8 col b
                pw = ps_tr_p.tile([128, LT, b_c], BF16, name="ptr")
                for i in range(LT):
                    nc.tensor.transpose(
                        pw[:, i, :],
                        pm_sb[0:b_c, i * 128:(i + 1) * 128],
                        idb[0:b_c, 0:b_c],
                    )
                nc.scalar.copy(W8[:, b, :, b], pw[:, :, b])

                # weighted sum: per-batch chunk accumulators drained to SBUF
                for n in range(NN):
                    ps_o = ps_out_p.tile([b_c, 512], F32, name="ps_out")
                    for i in range(LT):
                        nc.tensor.matmul(
                            ps_o[:, :],
                            lhsT=W8[:, b, i, :],
                            rhs=nats[i][:, n * 512:(n + 1) * 512],
                            start=(i == 0), stop=(i == LT - 1),
                        )
                    dstc = acc_sb[:, n * 512:(n + 1) * 512]
                    if b == 0:
                        nc.vector.tensor_copy(dstc, ps_o[:, :])
                    else:
                        nc.vector.tensor_add(dstc, dstc, ps_o[:, :])
                    if b == b_c - 1:
                        # finalize this chunk immediately (overlaps the tail)
                        nc.vector.tensor_scalar_mul(
                            out_sb[:, n * 512:(n + 1) * 512], dstc,
                            sinv_sb[:, :])
                        nc.sync.dma_start(
                            out_d[:, n * 512:(n + 1) * 512],
                            out_sb[:, n * 512:(n + 1) * 512])

            # ---- phase 1: qw64 = cq_aug @ W_slab, ReduceScatter over cores --
            # each core contracts its own KC k-tiles against all Ball batches
            qw64_sb = const.tile([Ball, d], F32)
            qwf_sb = const.tile([b_c, d], F32)
            accs = []
            for n in range(NN):
                pool = ps_lg_p if n % 2 == 0 else ps_out_p
                nm = "ps_lg" if n % 2 == 0 else "ps_out"
                accs.append(pool.tile([Ball, 512], F32, name=nm))
            for k in range(KC):
                wk = wpool.tile([128, d], BF16, name="wk")
                nc.sync.dma_start(
                    wk[:, :], w_d[k * 128:(k + 1) * 128, :])
                for n in range(NN):
                    nc.tensor.matmul(
                        accs[n],
                        lhsT=cqT_sb[:, k * Ball:(k + 1) * Ball],
                        rhs=wk[:, n * 512:(n + 1) * 512],
                        start=(k == 0),
                        stop=(k == KC - 1),
                    )
            # batch 0's context streams in behind the small W slab
            load(0)

            # W_attn is folded into W_aug on the host, so the PSUM chunks
            # already hold the local qw partials.  The qw bounce DMAs ride
            # the gpsimd queue so they never block the ctx stream.
            for n in range(NN):
                nc.vector.tensor_copy(qw64_sb[:, n * 512:(n + 1) * 512],
                                      accs[n])
            nc.gpsimd.dma_start(qw64_d[:, :], qw64_sb[:, :])
            nc.gpsimd.collective_compute(
                "ReduceScatter",
                mybir.AluOpType.add,
                replica_groups=[list(range(n_cores))],
                ins=[qw64_d[:, :].opt()],
                outs=[qwrs_d[:, :].opt()],
            )
            nc.gpsimd.dma_start(qwf_sb[:, :], qwrs_d[:, :])
            nc.vector.tensor_copy(qw_sb[:, :], qwf_sb[:, :])

            # qwT: [b_c, d] -> [128(d), DC*b_c] via PE transposes (bf16)
            pq = ps_tr_p.tile([128, DC * b_c], BF16, name="ptr")
            for j in range(DC):
                nc.tensor.transpose(
                    pq[:, j * b_c:(j + 1) * b_c],
                    qw_sb[:, j * 128:(j + 1) * 128],
                    idb[0:b_c, 0:b_c],
                )
            nc.vector.tensor_copy(qwT_sb[:, :], pq[:, :])

            for b in range(b_c):
                if b + 1 < b_c:
                    load(b + 1)
                attend(b)

    nc.compile()
    return nc


def host_prep(inputs, n_cores, b_c, l, d):
    """Slice/format the full inputs into per-core input maps."""
    step = int(np.asarray(inputs["step"]))
    context = np.asarray(inputs["context"], dtype=np.float32)
    question = np.asarray(inputs["question"], dtype=np.float32)
    control_state = np.asarray(inputs["control_state"], dtype=np.float32)
    q_mask = np.asarray(inputs["q_mask"])
    W_cq = np.asarray(inputs["W_cq"], dtype=np.float32)
    b_cq = np.asarray(inputs["b_cq"], dtype=np.float32)
    step_emb = np.asarray(inputs["step_emb"], dtype=np.float32)
    W_attn = np.asarray(inputs["W_attn"], dtype=np.float32)

    bf16 = ml_dtypes.bfloat16
    d2 = 2 * d
    KC = 5
    kpad = KC * 128 * n_cores   # 5120: padded contraction dim
    DC = d // 128

    bias = (b_cq + step_emb[step]).astype(np.float32)          # [d]
    cq = np.concatenate([control_state, question], axis=1)     # [B, 2d]
    Bfull = cq.shape[0]
    cq_aug = np.zeros((Bfull, kpad), dtype=np.float32)
    cq_aug[:, :d2] = cq
    cq_aug[:, d2] = 1.0
    # fold W_attn into the projection (parameter-only transform):
    # qw = (cq @ W_cq + bias) * W_attn = cq @ (W_cq * W_attn) + bias * W_attn
    W_aug = np.zeros((kpad, d), dtype=np.float32)
    W_aug[:d2] = W_cq * W_attn[None, :]
    W_aug[d2] = bias * W_attn
    W_aug_bf16 = W_aug.astype(bf16)

    def maskadd(m):
        out = np.zeros((128, l), dtype=np.float32)
        out[:b_c] = (m.astype(np.float32) - 1.0) * 10000.0
        return out.astype(bf16)

    ident_bf16 = np.eye(128, dtype=bf16)

    in_maps = []
    for c in range(n_cores):
        rows = slice(c * b_c, (c + 1) * b_c)
        kcols = slice(c * KC * 128, (c + 1) * KC * 128)
        # this core's k-slab of cq, for ALL batches: [128, KC, Bfull]
        cqT = np.ascontiguousarray(
            cq_aug[:, kcols].T.reshape(KC, 128, Bfull).transpose(1, 0, 2)
        ).astype(bf16)
        blob = np.concatenate(
            [ident_bf16, maskadd(q_mask[rows]),
             cqT.reshape(128, KC * Bfull)], axis=1)
        ctx_c = np.ascontiguousarray(context[rows]).astype(bf16)  # [b_c, l, d]
        # pre-transposed context: [b_c, DC, 128, l]
        ctxT_c = np.ascontiguousarray(
            ctx_c.transpose(0, 2, 1).reshape(b_c, DC, 128, l))
        in_maps.append({
            "ctx": ctx_c,
            "ctxT": ctxT_c,
            "blob": blob,
            "w_aug": np.ascontiguousarray(W_aug_bf16[kcols]),
        })
    return in_maps


_NC_CACHE = {}


def _get_nc(b_c, l, d, n_cores):
    key = (b_c, l, d, n_cores)
    if key not in _NC_CACHE:
        _NC_CACHE[key] = build_nc(b_c, l, d, n_cores)
    return _NC_CACHE[key]


def kernel(**inputs) -> np.ndarray:
    context = np.asarray(inputs["context"])
    Bfull, l, d = context.shape
    n_cores = N_CORES
    b_c = Bfull // n_cores

    nc = _get_nc(b_c, l, d, n_cores)
    in_maps = host_prep(inputs, n_cores, b_c, l, d)
    res = run_bass_kernel_spmd(nc, in_maps, list(range(n_cores)))
    out = np.concatenate([res.results[c]["out"] for c in range(n_cores)], axis=0)
    return out.astype(np.float32)


# revision 16
# speedup vs baseline: 1.4995x; 1.4995x over previous
"""Trainium2 Bass kernel for the ControlUnit problem.

Computation (per batch b):
    cq      = concat([control_state, question])            # [2D]
    cq_proj = cq @ W_cq + b_cq + step_emb[step]            # [D]
    qw      = cq_proj * W_attn                             # [D]
    logits  = context[b] @ qw  (+ b_attn, softmax-invariant, dropped)
    w       = softmax(where(mask, logits, -1e4))           # [L]
    out[b]  = w @ context[b]                               # [D]

Sharding: data-parallel over batch across 8 NeuronCores (8 batches/core);
params replicated.

Per-core kernel (all matmul operands bf16, fp32 PSUM accumulation):
  The host ships context twice: natural layout [l, d] (for the weighted
  sum, which contracts l) and pre-transposed [d, l] (for the logits,
  which contract d).  Shipping ctxT costs +16.8MB DMA but removes the
  512 PE transpose matmuls + 128 PSUM->SBUF drain copies an on-chip
  transpose needs; the per-exec wall time is dominated by instruction
  dispatch/sequencer occupancy, so trading DMA bytes for a ~5x smaller
  instruction stream is a net win on this runtime (measured).

  Phase 1: cq_proj matmul (bias folded via augmented ones column /
           bias row; [8,512] PSUM chunk accumulators), qw = cq_proj *
           W_attn, PE-transpose qw so d sits on partitions.
  Phase 2 (per batch): logits = one bf16 mask-bias matmul + 16 d-chunk
           matmuls of qwT against the shipped ctxT; one fused exp over
           [8,512] (no max subtraction - logits are ~N(0,1)); a
           predicated copy keeps the true softmax-numerator row; 4 PE
           transposes put the exp-weights l-on-partitions; the weighted
           sum accumulates in [8,512] PSUM chunks, DVE-added into an
           SBUF accumulator.  Denominators from one reduce_sum/
           reciprocal; final scale + output DMA run per 512-chunk.
"""
import numpy as np
import ml_dtypes
from contextlib import ExitStack

import concourse.bass as bass
import concourse.tile as tile
from concourse import bacc, mybir
from concourse.bass_utils import run_bass_kernel_spmd

F32 = mybir.dt.float32
BF16 = mybir.dt.bfloat16

N_CORES = 8
B, L, D = 64, 512, 2048


def build_nc(b_c, l, d, n_cores):
    """Build + compile the per-core Bass program (SPMD: same program on all
    cores, different data)."""
    d2a = 2 * d + 128          # augmented contraction dim (bias row block)
    KT = d2a // 128            # k-tiles for the cq_proj matmul
    LT = l // 128              # l-tiles per batch
    DC = d // 128              # 128-wide d-chunks
    NN = d // 512              # 512-wide n-chunks

    nc = bacc.Bacc("TRN2", target_bir_lowering=False, debug=False,
                   num_devices=n_cores)

    ctx_d = nc.dram_tensor("ctx", [b_c, l, d], BF16, kind="ExternalInput")
    ctxT_d = nc.dram_tensor("ctxT", [b_c, DC, 128, l], BF16,
                            kind="ExternalInput")
    BLOB = 128 + l + KT * b_c
    blob_d = nc.dram_tensor("blob", [128, BLOB], BF16, kind="ExternalInput")
    w_d = nc.dram_tensor("w_aug", [d2a, d], BF16, kind="ExternalInput")
    out_d = nc.dram_tensor("out", [b_c, d], F32, kind="ExternalOutput")

    Exp = mybir.ActivationFunctionType.Exp

    with tile.TileContext(nc) as tc:
        with ExitStack() as ctx:
            const = ctx.enter_context(tc.tile_pool(name="const", bufs=1))
            wpool = ctx.enter_context(tc.tile_pool(name="wpool", bufs=4))
            natpool = ctx.enter_context(tc.tile_pool(name="natpool", bufs=3))
            ctpool = ctx.enter_context(tc.tile_pool(name="ctpool", bufs=3))
            ps_lg_p = ctx.enter_context(tc.tile_pool(name="ps_lg_p", bufs=2,
                                                     space="PSUM"))
            ps_out_p = ctx.enter_context(tc.tile_pool(name="ps_out_p", bufs=2,
                                                      space="PSUM"))
            lgpool = ctx.enter_context(tc.tile_pool(name="lgpool", bufs=2))
            ps_tr_p = ctx.enter_context(tc.tile_pool(name="ps_tr_p", bufs=2,
                                                     space="PSUM"))

            # ---- constants / persistent tiles ----
            blob_sb = const.tile([128, BLOB], BF16)
            nc.sync.dma_start(blob_sb[:, :], blob_d[:, :])
            idb = blob_sb[:, 0:128]
            mask_sb = blob_sb[:, 128:128 + l]
            cqT_sb = blob_sb[:, 128 + l:]

            qw_sb = const.tile([b_c, d], BF16)
            qwT_sb = const.tile([128, DC * b_c], BF16)
            W8 = const.tile([128, b_c, LT, b_c], BF16)
            nc.gpsimd.memset(W8[:, :, :, :], 0.0)
            # per-batch exp row-sums land in column b; the true softmax
            # denominator for batch b is the diagonal element [b, b]
            ssum_all = const.tile([b_c, b_c], F32)
            ssum_sb = const.tile([b_c, 1], F32)
            sinv_sb = const.tile([b_c, 1], F32)
            out_sb = const.tile([b_c, d], F32)

            acc_sb = const.tile([b_c, d], F32)

            # ---- phase 2 helpers ----
            nat4s = {}
            ctbs = {}

            def load(b):
                # shipped pre-transposed context: [128 d-part, DC, l]
                ctb = ctpool.tile([128, DC, l], BF16, name="ctb")
                nc.sync.dma_start(
                    ctb[:, :, :],
                    ctxT_d[b].rearrange("c p n -> p c n"))
                ctbs[b] = ctb
                nat4 = natpool.tile([128, LT, d], BF16, name="nat4")
                nc.sync.dma_start(
                    nat4[:, :, :],
                    ctx_d[b].rearrange("(a p) n -> p a n", p=128))
                nat4s[b] = nat4

            def attend(b):
                nats = [nat4s[b][:, i, :] for i in range(LT)]
                ctb = ctbs[b]
                # logits for all queries vs this batch's context; row b is
                # real.  First matmul adds the -10000 mask bias.
                ps_lg = ps_lg_p.tile([b_c, l], F32, name="ps_lg")
                nc.tensor.matmul(
                    ps_lg[:, :],
                    lhsT=idb[:, 0:b_c],
                    rhs=mask_sb[:, :],
                    start=True, stop=False,
                )
                for j in range(DC):
                    nc.tensor.matmul(
                        ps_lg[:, :],
                        lhsT=qwT_sb[:, j * b_c:(j + 1) * b_c],
                        rhs=ctb[:, j, :],
                        start=False,
                        stop=(j == DC - 1),
                    )
                pm_sb = lgpool.tile([b_c, l], BF16, name="pm_sb")
                nc.scalar.activation(pm_sb[:, :], ps_lg[:, :], Exp,
                                     accum_out=ssum_all[:, b:b + 1])
                if b == b_c - 1:
                    # softmax denominators: diagonal of ssum_all (must be
                    # emitted before the per-chunk scale below so Tile
                    # orders the deps right)
                    nc.vector.tensor_mul(ssum_all[:, :], ssum_all[:, :],
                                         idb[0:b_c, 0:b_c])
                    nc.vector.reduce_sum(ssum_sb[:, :], ssum_all[:, :],
                                         axis=mybir.AxisListType.X)
                    nc.vector.reciprocal(sinv_sb[:, :], ssum_sb[:, :])

                # transpose exp-weights to l-on-partitions, drop into W8 col b
                pw = ps_tr_p.tile([128, LT, b_c], BF16, name="ptr")
                for i in range(LT):
                    nc.tensor.transpose(
                        pw[:, i, :],
                        pm_sb[0:b_c, i * 128:(i + 1) * 128],
                        idb[0:b_c, 0:b_c],
                    )
                nc.scalar.copy(W8[:, b, :, b], pw[:, :, b])

                # weighted sum: per-batch chunk accumulators drained to SBUF
                for n in range(NN):
                    ps_o = ps_out_p.tile([b_c, 512], F32, name="ps_out")
                    for i in range(LT):
                        nc.tensor.matmul(
                            ps_o[:, :],
                            lhsT=W8[:, b, i, :],
                            rhs=nats[i][:, n * 512:(n + 1) * 512],
                            start=(i == 0), stop=(i == LT - 1),
                        )
                    dstc = acc_sb[:, n * 512:(n + 1) * 512]
                    if b == 0:
                        nc.vector.tensor_copy(dstc, ps_o[:, :])
                    else:
                        nc.vector.tensor_add(dstc, dstc, ps_o[:, :])
                    if b == b_c - 1:
                        # finalize this chunk immediately (overlaps the tail)
                        nc.vector.tensor_scalar_mul(
                            out_sb[:, n * 512:(n + 1) * 512], dstc,
                            sinv_sb[:, :])
                        nc.sync.dma_start(
                            out_d[:, n * 512:(n + 1) * 512],
                            out_sb[:, n * 512:(n + 1) * 512])

            # batch 0's context streams in ahead of the W queue
            load(0)

            # ---- phase 1: cq_proj = cq_aug @ W_aug ----
            # chunked [b_c, 512] accumulators borrowed from the lg/out psum
            # pools (phase 1 finishes before batch-0 logits need them).
            accs = []
            for n in range(NN):
                pool = ps_lg_p if n % 2 == 0 else ps_out_p
                nm = "ps_lg" if n % 2 == 0 else "ps_out"
                accs.append(pool.tile([b_c, 512], F32, name=nm))
            # W DMA group sizes ramp up so the PE isn't starved at start
            kgs = []
            for g in (1, 1, 2):
                if sum(kgs) < KT:
                    kgs.append(min(g, KT - sum(kgs)))
            while sum(kgs) < KT:
                kgs.append(min(4, KT - sum(kgs)))
            kg = 0
            for kn in kgs:
                wk = wpool.tile([128, 4, d], BF16, name="wk")
                nc.sync.dma_start(
                    wk[:, 0:kn, :],
                    w_d[kg * 128:(kg + kn) * 128, :].rearrange(
                        "(a p) n -> p a n", p=128))
                for ki in range(kn):
                    k = kg + ki
                    for n in range(NN):
                        nc.tensor.matmul(
                            accs[n],
                            lhsT=cqT_sb[:, k * b_c:(k + 1) * b_c],
                            rhs=wk[:, ki, n * 512:(n + 1) * 512],
                            start=(k == 0),
                            stop=(k == KT - 1),
                        )
                kg += kn
            # W_attn is folded into W_aug on the host, so the PSUM chunks
            # already hold qw; drain straight to bf16
            for n in range(NN):
                nc.vector.tensor_copy(qw_sb[:, n * 512:(n + 1) * 512], accs[n])

            # qwT: [b_c, d] -> [128(d), DC*b_c] via PE transposes (bf16)
            pq = ps_tr_p.tile([128, DC * b_c], BF16, name="ptr")
            for j in range(DC):
                nc.tensor.transpose(
                    pq[:, j * b_c:(j + 1) * b_c],
                    qw_sb[:, j * 128:(j + 1) * 128],
                    idb[0:b_c, 0:b_c],
                )
            nc.vector.tensor_copy(qwT_sb[:, :], pq[:, :])

            for b in range(b_c):
                if b + 1 < b_c:
                    load(b + 1)
                attend(b)

    nc.compile()
    return nc


def host_prep(inputs, n_cores, b_c, l, d):
    """Slice/format the full inputs into per-core input maps."""
    step = int(np.asarray(inputs["step"]))
    context = np.asarray(inputs["context"], dtype=np.float32)
    question = np.asarray(inputs["question"], dtype=np.float32)
    control_state = np.asarray(inputs["control_state"], dtype=np.float32)
    q_mask = np.asarray(inputs["q_mask"])
    W_cq = np.asarray(inputs["W_cq"], dtype=np.float32)
    b_cq = np.asarray(inputs["b_cq"], dtype=np.float32)
    step_emb = np.asarray(inputs["step_emb"], dtype=np.float32)
    W_attn = np.asarray(inputs["W_attn"], dtype=np.float32)

    bf16 = ml_dtypes.bfloat16
    d2 = 2 * d
    d2a = d2 + 128
    KT = d2a // 128
    DC = d // 128

    bias = (b_cq + step_emb[step]).astype(np.float32)          # [d]
    cq = np.concatenate([control_state, question], axis=1)     # [B, 2d]
    Bfull = cq.shape[0]
    cq_aug = np.zeros((Bfull, d2a), dtype=np.float32)
    cq_aug[:, :d2] = cq
    cq_aug[:, d2] = 1.0
    # fold W_attn into the projection (parameter-only transform):
    # qw = (cq @ W_cq + bias) * W_attn = cq @ (W_cq * W_attn) + bias * W_attn
    W_aug = np.zeros((d2a, d), dtype=np.float32)
    W_aug[:d2] = W_cq * W_attn[None, :]
    W_aug[d2] = bias * W_attn
    W_aug_bf16 = W_aug.astype(bf16)

    def maskadd(m):
        out = np.zeros((128, l), dtype=np.float32)
        out[:b_c] = (m.astype(np.float32) - 1.0) * 10000.0
        return out.astype(bf16)

    ident_bf16 = np.eye(128, dtype=bf16)

    in_maps = []
    for c in range(n_cores):
        rows = slice(c * b_c, (c + 1) * b_c)
        cqT = np.ascontiguousarray(
            cq_aug[rows].T.reshape(KT, 128, b_c).transpose(1, 0, 2)
        ).astype(bf16)                                          # [128, KT, b_c]
        blob = np.concatenate(
            [ident_bf16, maskadd(q_mask[rows]),
             cqT.reshape(128, KT * b_c)], axis=1)
        ctx_c = np.ascontiguousarray(context[rows]).astype(bf16)  # [b_c, l, d]
        # pre-transposed context: [b_c, DC, 128, l]
        ctxT_c = np.ascontiguousarray(
            ctx_c.transpose(0, 2, 1).reshape(b_c, DC, 128, l))
        in_maps.append({
            "ctx": ctx_c,
            "ctxT": ctxT_c,
            "blob": blob,
            "w_aug": W_aug_bf16,
        })
    return in_maps


_NC_CACHE = {}


def _get_nc(b_c, l, d, n_cores):
    key = (b_c, l, d, n_cores)
    if key not in _NC_CACHE:
        _NC_CACHE[key] = build_nc(b_c, l, d, n_cores)
    return _NC_CACHE[key]


def kernel(**inputs) -> np.ndarray:
    context = np.asarray(inputs["context"])
    Bfull, l, d = context.shape
    n_cores = N_CORES
    b_c = Bfull // n_cores

    nc = _get_nc(b_c, l, d, n_cores)
    in_maps = host_prep(inputs, n_cores, b_c, l, d)
    res = run_bass_kernel_spmd(nc, in_maps, list(range(n_cores)))
    out = np.concatenate([res.results[c]["out"] for c in range(n_cores)], axis=0)
    return out.astype(np.float32)
